# revision 19
# baseline (speedup 1.0000x reference)
"""Trainium2 Bass kernel for nn_RahmanDynamicNet:
conv(1->20,(34,5)) -> BN(eval) -> sigmoid -> ParametricLIF -> linear(20->1)
-> sigmoid -> ParametricLIF -> [B,T] float32.

Self-contained: takes FULL inputs, shards T across 8 NeuronCores (SPMD, no
collectives), returns the FULL [B,T] output.

Math (same identities as the tuned v1 baseline):
  - Conv output feeds sigmoid => y in (0,1); LIF state v stays << VTH=1000,
    so spikes never fire and both LIF layers are pure EMAs ->
    tensor_tensor_scan (no T-loop).
  - EMA commutes with the linear readout: lin(EMA(u)) = EMA(lin(u)).
  - T sharded with a 25-step EMA warmup per core (state error ~0.5^25);
    core 0 is exact (scan decay zeroed at the warmup boundary column).

v2 perf structure (vs the v1 baseline at ~32us/rep):
  - lhsT patches are pre-expanded on HOST into xe[row, chunk, block, b],
    so each segment loads with ONE DMA of large contiguous descriptors
    (v1: 44 small DMAs/rep serialized ~27us of HWDGE descriptor-gen).
  - S=25 outputs per block: each block's 500 conv columns exactly fill one
    PSUM bank, so the sigmoid is a single-bank contiguous ACT read with
    bf16 out (multi-bank bf16 ACT reads silently corrupt); 172-cycle PSUM
    bubble amortized over 500 cols.
  - WARM=25, TL=525 (vs 576): 9% less of everything.
  - Conv+BN as band-sparse matmuls: 8 chunks of 127 rows, per-column
    start=True exactly once via split start/accum ranges (15 matmuls and
    1160 streamed cols per 25 t vs v1's 50/t).
  - h-contraction: bf16 tensor_mul by tiled lin_w*sw1 + pairwise add tree
    20->10->5 (DVE 2x mode), 10->5 level on the idle GPSIMD, final 5->1
    tensor_reduce + both EMA scans on DVE, z-sigmoid on ACT.
  - No output copy: v EMA is stored unscaled, DMA'd straight out; the
    final *sw2 scale happens on host after the gather.
"""
import numpy as np
from contextlib import ExitStack
import sys

sys.path.insert(0, "/opt/trn_rl_repo")

import concourse.bass as bass
import concourse.bacc as bacc
import concourse.tile as tile
from concourse import mybir
from concourse.bass_utils import run_bass_kernel_spmd
import ml_dtypes

BF16 = ml_dtypes.bfloat16

B, F, T, H, K = 128, 34, 4000, 20, 5
NCORES = 8
S = 25           # outputs per block (500 cols = one PSUM bank)
JW = S + 4       # patch window (taps)
FA = F + 1       # augmented channels (x + ones)
ROWS = JW * FA   # 1015
NCHUNK = 8
CHROWS = 127     # 8*127 = 1016 (1 zero-pad row)
NB = 21          # blocks per core
NSEG = 3
BPS = NB // NSEG     # 7 blocks per segment
TL = NB * S          # 525
WARM = 25
TO = T // NCORES     # 500
PADL = 48
BCOLS = S * H        # 500
SEGT = TL // NSEG    # 175 t per segment
BN_EPS = 1e-5

_DT = mybir.dt

# (chunk, (col0, col1), start) — per-block matmul list. start=True clears
# has_written for the WHOLE psum bank (not just its column range), so the
# bank gets exactly ONE start: chunk 0 streams full width (every column's
# BN-shift bias lives on chunk-0's ones row), chunks 1..7 accumulate their
# x-band column ranges only.
MMLIST = [
    (0, (0, 500), True),
    (1, (0, 160), False),
    (2, (60, 220), False),
    (3, (120, 300), False),
    (4, (200, 380), False),
    (5, (280, 440), False),
    (6, (340, 500), False),
    (7, (420, 500), False),
]


def _sigmoid(v):
    return 1.0 / (1.0 + np.exp(-v))


def build_nc(sw1, sw2, reps=1):
    nc = bacc.Bacc()
    # host-pre-expanded lhsT patches: xe[p, c, ibl, b] = x-patch row
    # (127c+p) of block ibl; (ibl, b) contiguous -> big-descriptor DMAs.
    xep = nc.declare_dram_parameter("xe", [CHROWS, NCHUNK, NB, B],
                                    _DT.float8e3, isOutput=False)
    wcp = nc.declare_dram_parameter("wc", [CHROWS, NCHUNK, BCOLS], _DT.bfloat16,
                                    isOutput=False)
    wrp = nc.declare_dram_parameter("wrep", [B, BPS * BCOLS], _DT.bfloat16,
                                    isOutput=False)
    cst = nc.declare_dram_parameter("consts", [B, 4], _DT.float32, isOutput=False)
    d0ap = nc.declare_dram_parameter("d0a", [B, TL], _DT.float32, isOutput=False)
    d0bp = nc.declare_dram_parameter("d0b", [B, TL], _DT.float32, isOutput=False)
    outp = nc.declare_dram_parameter("out", [B, TO], _DT.float32, isOutput=True)

    with ExitStack() as ctx:
        tc = ctx.enter_context(tile.TileContext(nc))
        singles = ctx.enter_context(tc.tile_pool(name="singles", bufs=1))
        xp = ctx.enter_context(tc.tile_pool(name="xp", bufs=2))
        pp = ctx.enter_context(tc.tile_pool(name="pp", bufs=4, space="PSUM"))
        up = ctx.enter_context(tc.tile_pool(name="up", bufs=2))
        ump = ctx.enter_context(tc.tile_pool(name="ump", bufs=2))
        um2p = ctx.enter_context(tc.tile_pool(name="um2p", bufs=2))
        um3p = ctx.enter_context(tc.tile_pool(name="um3p", bufs=2))

        wc_sb = singles.tile([CHROWS, NCHUNK, BCOLS], _DT.bfloat16)
        nc.sync.dma_start(out=wc_sb, in_=wcp[:, :, :])
        wrep_sb = singles.tile([B, BPS * BCOLS], _DT.bfloat16)
        nc.sync.dma_start(out=wrep_sb, in_=wrp[:, :])
        cst_sb = singles.tile([B, 4], _DT.float32)
        nc.sync.dma_start(out=cst_sb, in_=cst[:, :])
        d0a_sb = singles.tile([B, TL], _DT.float32)
        nc.sync.dma_start(out=d0a_sb, in_=d0ap[:, :])
        d0b_sb = singles.tile([B, TL], _DT.float32)
        nc.sync.dma_start(out=d0b_sb, in_=d0bp[:, :])

        p_sb = singles.tile([B, TL], _DT.float32)
        q_sb = singles.tile([B, TL], _DT.float32)
        z_sb = singles.tile([B, TL], _DT.float32)
        v_sb = singles.tile([B, TL], _DT.float32)

        for _rep in range(reps):
            for seg in range(NSEG):
                # one contiguous-descriptor DMA per segment: all 8 chunks
                # x 7 blocks of pre-expanded lhsT patches.
                xb = xp.tile([CHROWS, NCHUNK, BPS, B], _DT.float8e3)
                eng = nc.sync if (seg % 2 == 0) else nc.scalar
                eng.dma_start(out=xb[:, :, :, :],
                              in_=xep[:, :, BPS * seg:BPS * (seg + 1), :])

                u_g = up.tile([B, BPS * BCOLS], _DT.bfloat16)
                for e in range(BPS):
                    psb = pp.tile([B, 512], _DT.float32)
                    for mi, (c, (a0, a1), st) in enumerate(MMLIST):
                        nc.tensor.matmul(
                            psb[:, a0:a1], xb[:, c, e, :],
                            wc_sb[:, c, a0:a1],
                            start=st, stop=(mi == len(MMLIST) - 1),
                            skip_group_check=True,
                        )
                    nc.scalar.activation(
                        out=u_g[:, BCOLS * e:BCOLS * (e + 1)],
                        in_=psb[:, 0:BCOLS],
                        func=mybir.ActivationFunctionType.Sigmoid,
                    )

                # h-contraction: p[:, t] = sum_h u*wrep; mul + 20->10 add
                # on DVE (2x bf16), 10->5 on Pool, 5->1 reduce on DVE.
                um = ump.tile([B, BPS * BCOLS], _DT.bfloat16)
                nc.vector.tensor_mul(um[:, :], u_g[:, :], wrep_sb[:, :])
                umv = um.rearrange("p (t h) -> p t h", h=H)
                um2 = um2p.tile([B, SEGT * 10], _DT.bfloat16)
                um2v = um2.rearrange("p (t h) -> p t h", h=10)
                nc.vector.tensor_add(um2v[:, :, :], umv[:, :, 0:10],
                                     umv[:, :, 10:20])
                um3 = um3p.tile([B, SEGT * 5], _DT.bfloat16)
                um3v = um3.rearrange("p (t h) -> p t h", h=5)
                nc.gpsimd.tensor_add(um3v[:, :, :], um2v[:, :, 0:5],
                                     um2v[:, :, 5:10])
                nc.vector.tensor_reduce(
                    out=p_sb[:, SEGT * seg:SEGT * (seg + 1)],
                    in_=um3v[:, :, :],
                    axis=mybir.AxisListType.X, op=mybir.AluOpType.add,
                )

                # chained EMA scans + output tail for this segment
                s0, s1 = SEGT * seg, SEGT * (seg + 1)
                nc.vector.tensor_tensor_scan(
                    out=q_sb[:, s0:s1], data0=d0a_sb[:, s0:s1],
                    data1=p_sb[:, s0:s1],
                    initial=(0.0 if seg == 0 else q_sb[:, s0 - 1:s0]),
                    op0=mybir.AluOpType.mult, op1=mybir.AluOpType.add,
                )
                nc.scalar.activation(
                    out=z_sb[:, s0:s1], in_=q_sb[:, s0:s1],
                    func=mybir.ActivationFunctionType.Sigmoid,
                    bias=cst_sb[:, 2:3],
                )
                nc.vector.tensor_tensor_scan(
                    out=v_sb[:, s0:s1], data0=d0b_sb[:, s0:s1],
                    data1=z_sb[:, s0:s1],
                    initial=(0.0 if seg == 0 else v_sb[:, s0 - 1:s0]),
                    op0=mybir.AluOpType.mult, op1=mybir.AluOpType.add,
                )
                c0 = max(0, s0 - WARM)
                c1 = min(TO, s1 - WARM)
                nc.sync.dma_start(out=outp[:, c0:c1],
                                  in_=v_sb[:, WARM + c0:WARM + c1])
    nc.compile()
    return nc


def prep(x, conv_w, conv_b, bn_gamma, bn_beta, bn_mean, bn_var,
         lin_w, lin_b, w1, w2):
    x = np.asarray(x, np.float32)
    inv = (np.asarray(bn_gamma, np.float32)
           / np.sqrt(np.asarray(bn_var, np.float32) + BN_EPS))
    shift = (np.asarray(conv_b, np.float32)
             - np.asarray(bn_mean, np.float32)) * inv \
        + np.asarray(bn_beta, np.float32)
    sw1 = float(_sigmoid(np.float32(np.asarray(w1))))
    sw2 = float(_sigmoid(np.float32(np.asarray(w2))))
    linb = float(np.asarray(lin_b, np.float32).reshape(-1)[0])
    lw = np.asarray(lin_w, np.float32).reshape(-1)

    GT = PADL + T + 40
    x_aug = np.zeros((GT, FA, B), np.float32)
    x_aug[PADL:PADL + T, :F, :] = x[:, 0].transpose(2, 1, 0)
    x_aug[PADL:PADL + T, F, :] = 1.0
    x_aug_f8 = x_aug.astype(ml_dtypes.float8_e3m4)

    # patch-row expansion indices: row r = 127c + p -> (j, ch)
    r_idx = np.arange(NCHUNK * CHROWS)
    j_idx = r_idx // FA
    ch_idx = r_idx % FA
    ibl_off = S * np.arange(NB)

    cw = np.asarray(conv_w, np.float32)[:, 0]  # [H,F,K]
    Wf = np.zeros((NCHUNK * CHROWS, BCOLS), np.float32)
    for i in range(S):
        for k in range(K):
            j = i + k
            Wf[j * FA:j * FA + F, i * H:(i + 1) * H] = \
                (cw[:, :, k] * inv[:, None]).T
        # all BN-shift biases on chunk-0's j=2 ones row (row 104): keeps the
        # single full-width start=True on chunk 0, and t=g0+25*ibl+2 stays
        # inside the real ones region for every core/block that matters.
        Wf[2 * FA + F, i * H:(i + 1) * H] = shift
    wc = np.ascontiguousarray(
        Wf.reshape(NCHUNK, CHROWS, BCOLS).transpose(1, 0, 2)).astype(BF16)

    wr = np.tile(lw * sw1, BPS * S).astype(BF16)
    wrep = np.ascontiguousarray(np.broadcast_to(wr, (B, BPS * BCOLS)))

    consts = np.zeros((B, 4), np.float32)
    consts[:, 0] = 1.0 - sw1
    consts[:, 1] = 1.0 - sw2
    consts[:, 2] = linb

    d0a = np.full((B, TL), 1.0 - sw1, np.float32)
    d0b = np.full((B, TL), 1.0 - sw2, np.float32)
    d0a0 = d0a.copy(); d0a0[:, WARM] = 0.0
    d0b0 = d0b.copy(); d0b0[:, WARM] = 0.0

    in_maps = []
    for c in range(NCORES):
        g0 = 500 * c + PADL - WARM - 2
        t_idx = g0 + ibl_off[None, :] + j_idx[:, None]      # [1016, 21]
        xe = x_aug_f8[t_idx, ch_idx[:, None], :]            # [1016, 21, 128]
        xe = np.ascontiguousarray(
            xe.reshape(NCHUNK, CHROWS, NB, B).transpose(1, 0, 2, 3))
        in_maps.append({"xe": xe, "wc": wc, "wrep": wrep, "consts": consts,
                        "d0a": d0a0 if c == 0 else d0a,
                        "d0b": d0b0 if c == 0 else d0b})
    return in_maps, sw1, sw2


_NC_CACHE = {}


def kernel(**inputs):
    in_maps, sw1, sw2 = prep(**inputs)
    key = (round(sw1, 9), round(sw2, 9))
    if key not in _NC_CACHE:
        _NC_CACHE[key] = build_nc(sw1, sw2)
    nc = _NC_CACHE[key]
    res = run_bass_kernel_spmd(nc, in_maps, list(range(NCORES)))
    outs = [np.asarray(res.results[c]["out"], np.float32)
            for c in range(NCORES)]
    return np.float32(sw2) * np.concatenate(outs, axis=1)


# revision 21
# speedup vs baseline: 3.7687x; 3.7687x over previous
"""Trainium2 Bass kernel for nn_RahmanDynamicNet:
conv(1->20,(34,5)) -> BN(eval) -> sigmoid -> ParametricLIF -> linear(20->1)
-> sigmoid -> ParametricLIF -> [B,T] float32.

Self-contained: takes FULL inputs, shards T across 8 NeuronCores (SPMD, no
collectives), returns the FULL [B,T] output.

Math (same identities as the tuned v1 baseline):
  - Conv output feeds sigmoid => y in (0,1); LIF state v stays << VTH=1000,
    so spikes never fire and both LIF layers are pure EMAs ->
    tensor_tensor_scan (no T-loop).
  - EMA commutes with the linear readout: lin(EMA(u)) = EMA(lin(u)).
  - T sharded with a 25-step EMA warmup per core (state error ~0.5^25);
    core 0 is exact (scan decay zeroed at the warmup boundary column).

Perf structure (HW-measured findings, vs the v1 baseline at ~32us/rep):
  - S=25 outputs per block: each block's 500 conv columns exactly fill one
    PSUM bank, so the sigmoid is a single-bank contiguous ACT read with
    bf16 out (multi-bank bf16 ACT reads silently corrupt); the 172-cycle
    PSUM bubble amortizes over 500 cols (v1: 480/3-block groups).
  - matmul start=True clears has_written for the WHOLE psum bank, so each
    block issues ONE full-width start (chunk 0, which also carries every
    column's BN-shift bias on its j=2 ones row) + 9 band accumulates.
  - lhsT patch rows (j,ch) have uniform stride B in the [t, ch, b] fp8
    layout, so each 105-row chunk loads with ONE windowed DMA
    [[B,105],[25*st,21],[1,B]]. Measured: this AP shape sustains
    ~320-360 GB/s, while wide flat APs (small middle dim) collapse to
    ~25 GB/s. 10 load DMAs + 3 store DMAs per rep (v1: 44).
  - WARM=25, TL=525 (vs 576): 9% less of everything.
  - h-contraction: bf16 tensor_mul by tiled lin_w*sw1 + pairwise add tree
    20->10->5; the 10->5 level runs on the idle GPSIMD, the rest plus the
    EMA scans on DVE (2x bf16 perf mode), z-sigmoid on ACT.
  - No output copy: v EMA is stored unscaled, DMA'd straight out; the
    final *sw2 scale happens on host after the gather.
"""
import numpy as np
from contextlib import ExitStack
import sys

sys.path.insert(0, "/opt/trn_rl_repo")

import concourse.bass as bass
import concourse.bacc as bacc
import concourse.tile as tile
from concourse import mybir
from concourse.bass_utils import run_bass_kernel_spmd
import ml_dtypes

BF16 = ml_dtypes.bfloat16

B, F, T, H, K = 128, 34, 4000, 20, 5
NCORES = 8
S = 25           # outputs per block (500 cols = one PSUM bank)
JW = S + 4       # patch window (taps)
FA = F + 1       # augmented channels (x + ones)
ROWS = JW * FA   # 1015
NCHUNK = 10
CHROWS = 105     # 10*105 = 1050 (35 zero-pad rows)
NB = 21          # blocks per core
UGRP = 7         # blocks per h-contraction group
NG = NB // UGRP  # 3 groups
TL = NB * S      # 525
WARM = 25
TO = T // NCORES     # 500
PADL = 48
XT_W = 530
BCOLS = S * H        # 500
GT_T = UGRP * S      # 175 t per group
BN_EPS = 1e-5

_DT = mybir.dt

# (chunk, (col0, col1)) — per-block matmul list. start=True clears
# has_written for the WHOLE psum bank, so chunk 0 is the single start:
# full width, with every column's BN-shift bias on its ones row; chunks
# 1..9 accumulate their x-band column ranges.
MMLIST = [
    (0, (0, 500)),
    (1, (0, 120)),
    (2, (40, 180)),
    (3, (100, 240)),
    (4, (160, 300)),
    (5, (220, 360)),
    (6, (280, 420)),
    (7, (340, 480)),
    (8, (400, 500)),
    (9, (460, 500)),
]


def _sigmoid(v):
    return 1.0 / (1.0 + np.exp(-v))


def build_nc(sw1, sw2, reps=1):
    nc = bacc.Bacc()
    xt = nc.declare_dram_parameter("xt", [XT_W, FA, B], _DT.float8e3,
                                   isOutput=False)
    wcp = nc.declare_dram_parameter("wc", [CHROWS, NCHUNK, BCOLS], _DT.bfloat16,
                                    isOutput=False)
    wrp = nc.declare_dram_parameter("wrep", [B, UGRP * BCOLS], _DT.bfloat16,
                                    isOutput=False)
    cst = nc.declare_dram_parameter("consts", [B, 4], _DT.float32, isOutput=False)
    d0ap = nc.declare_dram_parameter("d0a", [B, TL], _DT.float32, isOutput=False)
    d0bp = nc.declare_dram_parameter("d0b", [B, TL], _DT.float32, isOutput=False)
    outp = nc.declare_dram_parameter("out", [B, TO], _DT.float32, isOutput=True)

    st_t = FA * B  # xt t-stride in elements

    with ExitStack() as ctx:
        tc = ctx.enter_context(tile.TileContext(nc))
        singles = ctx.enter_context(tc.tile_pool(name="singles", bufs=1))
        xp = ctx.enter_context(tc.tile_pool(name="xp", bufs=2))
        pp = ctx.enter_context(tc.tile_pool(name="pp", bufs=4, space="PSUM"))
        up = ctx.enter_context(tc.tile_pool(name="up", bufs=2))
        ump = ctx.enter_context(tc.tile_pool(name="ump", bufs=2))
        um2p = ctx.enter_context(tc.tile_pool(name="um2p", bufs=2))
        um3p = ctx.enter_context(tc.tile_pool(name="um3p", bufs=2))

        wc_sb = singles.tile([CHROWS, NCHUNK, BCOLS], _DT.bfloat16)
        nc.sync.dma_start(out=wc_sb, in_=wcp[:, :, :])
        wrep_sb = singles.tile([B, UGRP * BCOLS], _DT.bfloat16)
        nc.sync.dma_start(out=wrep_sb, in_=wrp[:, :])
        cst_sb = singles.tile([B, 4], _DT.float32)
        nc.sync.dma_start(out=cst_sb, in_=cst[:, :])
        d0a_sb = singles.tile([B, TL], _DT.float32)
        nc.sync.dma_start(out=d0a_sb, in_=d0ap[:, :])
        d0b_sb = singles.tile([B, TL], _DT.float32)
        nc.sync.dma_start(out=d0b_sb, in_=d0bp[:, :])

        p_sb = singles.tile([B, TL], _DT.float32)
        q_sb = singles.tile([B, TL], _DT.float32)
        z_sb = singles.tile([B, TL], _DT.float32)
        v_sb = singles.tile([B, TL], _DT.float32)

        xt_ap = xt[:, :, :]

        for _rep in range(reps):
            # one windowed DMA per 105-row chunk loads the whole rep's
            # lhsT patches: [105 rows, 21 blocks, 128 b] (~320 GB/s shape).
            xb = xp.tile([CHROWS, NCHUNK, NB, B], _DT.float8e3)
            for c in range(NCHUNK):
                src = bass.AP(
                    tensor=xt_ap.tensor,
                    offset=c * CHROWS * B,
                    ap=[[B, CHROWS], [S * st_t, NB], [1, B]],
                )
                eng = nc.sync if (c % 2 == 0) else nc.scalar
                eng.dma_start(out=xb[:, c, :, :], in_=src)

            for g in range(NG):
                u_g = up.tile([B, UGRP * BCOLS], _DT.bfloat16)
                for e in range(UGRP):
                    ibl = UGRP * g + e
                    psb = pp.tile([B, 512], _DT.float32)
                    for mi, (c, (a0, a1)) in enumerate(MMLIST):
                        nc.tensor.matmul(
                            psb[:, a0:a1], xb[:, c, ibl, :],
                            wc_sb[:, c, a0:a1],
                            start=(mi == 0), stop=(mi == len(MMLIST) - 1),
                            skip_group_check=True,
                        )
                    nc.scalar.activation(
                        out=u_g[:, BCOLS * e:BCOLS * (e + 1)],
                        in_=psb[:, 0:BCOLS],
                        func=mybir.ActivationFunctionType.Sigmoid,
                    )

                # h-contraction: p[:, t] = sum_h u*wrep; mul + 20->10 add
                # on DVE (2x bf16), 10->5 on Pool, 5->1 reduce on DVE.
                um = ump.tile([B, UGRP * BCOLS], _DT.bfloat16)
                nc.vector.tensor_mul(um[:, :], u_g[:, :], wrep_sb[:, :])
                umv = um.rearrange("p (t h) -> p t h", h=H)
                um2 = um2p.tile([B, GT_T * 10], _DT.bfloat16)
                um2v = um2.rearrange("p (t h) -> p t h", h=10)
                nc.vector.tensor_add(um2v[:, :, :], umv[:, :, 0:10],
                                     umv[:, :, 10:20])
                um3 = um3p.tile([B, GT_T * 5], _DT.bfloat16)
                um3v = um3.rearrange("p (t h) -> p t h", h=5)
                nc.gpsimd.tensor_add(um3v[:, :, :], um2v[:, :, 0:5],
                                     um2v[:, :, 5:10])
                nc.vector.tensor_reduce(
                    out=p_sb[:, GT_T * g:GT_T * (g + 1)],
                    in_=um3v[:, :, :],
                    axis=mybir.AxisListType.X, op=mybir.AluOpType.add,
                )

                # chained EMA scans + output tail for this group
                s0, s1 = GT_T * g, GT_T * (g + 1)
                nc.vector.tensor_tensor_scan(
                    out=q_sb[:, s0:s1], data0=d0a_sb[:, s0:s1],
                    data1=p_sb[:, s0:s1],
                    initial=(0.0 if g == 0 else q_sb[:, s0 - 1:s0]),
                    op0=mybir.AluOpType.mult, op1=mybir.AluOpType.add,
                )
                nc.scalar.activation(
                    out=z_sb[:, s0:s1], in_=q_sb[:, s0:s1],
                    func=mybir.ActivationFunctionType.Sigmoid,
                    bias=cst_sb[:, 2:3],
                )
                nc.vector.tensor_tensor_scan(
                    out=v_sb[:, s0:s1], data0=d0b_sb[:, s0:s1],
                    data1=z_sb[:, s0:s1],
                    initial=(0.0 if g == 0 else v_sb[:, s0 - 1:s0]),
                    op0=mybir.AluOpType.mult, op1=mybir.AluOpType.add,
                )
                c0 = max(0, s0 - WARM)
                c1 = min(TO, s1 - WARM)
                nc.sync.dma_start(out=outp[:, c0:c1],
                                  in_=v_sb[:, WARM + c0:WARM + c1])
    nc.compile()
    return nc


def prep(x, conv_w, conv_b, bn_gamma, bn_beta, bn_mean, bn_var,
         lin_w, lin_b, w1, w2):
    x = np.asarray(x, np.float32)
    inv = (np.asarray(bn_gamma, np.float32)
           / np.sqrt(np.asarray(bn_var, np.float32) + BN_EPS))
    shift = (np.asarray(conv_b, np.float32)
             - np.asarray(bn_mean, np.float32)) * inv \
        + np.asarray(bn_beta, np.float32)
    sw1 = float(_sigmoid(np.float32(np.asarray(w1))))
    sw2 = float(_sigmoid(np.float32(np.asarray(w2))))
    linb = float(np.asarray(lin_b, np.float32).reshape(-1)[0])
    lw = np.asarray(lin_w, np.float32).reshape(-1)

    GT = PADL + T + 40
    x_aug = np.zeros((GT, FA, B), np.float32)
    x_aug[PADL:PADL + T, :F, :] = x[:, 0].transpose(2, 1, 0)
    x_aug[PADL:PADL + T, F, :] = 1.0
    x_aug_f8 = x_aug.astype(ml_dtypes.float8_e3m4)

    cw = np.asarray(conv_w, np.float32)[:, 0]  # [H,F,K]
    Wf = np.zeros((NCHUNK * CHROWS, BCOLS), np.float32)
    for i in range(S):
        for k in range(K):
            j = i + k
            Wf[j * FA:j * FA + F, i * H:(i + 1) * H] = \
                (cw[:, :, k] * inv[:, None]).T
        # all BN-shift biases on chunk-0's j=2 ones row (row 104): keeps the
        # single full-width start=True on chunk 0, and t=g0+25*ibl+2 stays
        # inside the real ones region for every core/block that matters.
        Wf[2 * FA + F, i * H:(i + 1) * H] = shift
    wc = np.ascontiguousarray(
        Wf.reshape(NCHUNK, CHROWS, BCOLS).transpose(1, 0, 2)).astype(BF16)

    wr = np.tile(lw * sw1, UGRP * S).astype(BF16)
    wrep = np.ascontiguousarray(np.broadcast_to(wr, (B, UGRP * BCOLS)))

    consts = np.zeros((B, 4), np.float32)
    consts[:, 0] = 1.0 - sw1
    consts[:, 1] = 1.0 - sw2
    consts[:, 2] = linb

    d0a = np.full((B, TL), 1.0 - sw1, np.float32)
    d0b = np.full((B, TL), 1.0 - sw2, np.float32)
    d0a0 = d0a.copy(); d0a0[:, WARM] = 0.0
    d0b0 = d0b.copy(); d0b0[:, WARM] = 0.0

    in_maps = []
    for c in range(NCORES):
        g0 = 500 * c + PADL - WARM - 2
        xtc = np.ascontiguousarray(x_aug_f8[g0:g0 + XT_W, :, :])
        in_maps.append({"xt": xtc, "wc": wc, "wrep": wrep, "consts": consts,
                        "d0a": d0a0 if c == 0 else d0a,
                        "d0b": d0b0 if c == 0 else d0b})
    return in_maps, sw1, sw2


_NC_CACHE = {}


def kernel(**inputs):
    in_maps, sw1, sw2 = prep(**inputs)
    key = (round(sw1, 9), round(sw2, 9))
    if key not in _NC_CACHE:
        _NC_CACHE[key] = build_nc(sw1, sw2)
    nc = _NC_CACHE[key]
    res = run_bass_kernel_spmd(nc, in_maps, list(range(NCORES)))
    outs = [np.asarray(res.results[c]["out"], np.float32)
            for c in range(NCORES)]
    return np.float32(sw2) * np.concatenate(outs, axis=1)


# revision 25
# speedup vs baseline: 5.2380x; 1.3899x over previous
"""Trainium2 Bass kernel for nn_RahmanDynamicNet:
conv(1->20,(34,5)) -> BN(eval) -> sigmoid -> ParametricLIF -> linear(20->1)
-> sigmoid -> ParametricLIF -> [B,T] float32.

Self-contained: takes FULL inputs, shards T across 8 NeuronCores (SPMD, no
collectives), returns the FULL [B,T] output.

Math (same identities as the tuned v1 baseline):
  - Conv output feeds sigmoid => y in (0,1); LIF state v stays << VTH=1000,
    so spikes never fire and both LIF layers are pure EMAs ->
    tensor_tensor_scan (no T-loop).
  - EMA commutes with the linear readout: lin(EMA(u)) = EMA(lin(u)).
  - T sharded with a 25-step EMA warmup per core (state error ~0.5^25);
    core 0 is exact (scan decay zeroed at the warmup boundary column).

Perf structure (HW-measured findings, vs the v1 baseline at ~32us/rep):
  - S=25 outputs per block: each block's 500 conv columns exactly fill one
    PSUM bank, so the sigmoid is a single-bank contiguous ACT read with
    bf16 out (multi-bank bf16 ACT reads silently corrupt); the 172-cycle
    PSUM bubble amortizes over 500 cols (v1: 480/3-block groups).
  - matmul start=True clears has_written for the WHOLE psum bank, so each
    block issues ONE full-width start (chunk 0, which also carries every
    column's BN-shift bias on its j=2 ones row) + 9 band accumulates.
  - lhsT patch rows (j,ch) have uniform stride B in the [t, ch, b] fp8
    layout, so each 105-row chunk loads with ONE windowed DMA
    [[B,105],[25*st,21],[1,B]]. Measured: this AP shape sustains
    ~320-360 GB/s, while wide flat APs (small middle dim) collapse to
    ~25 GB/s. 10 load DMAs + 3 store DMAs per rep (v1: 44).
  - WARM=25, TL=525 (vs 576): 9% less of everything.
  - h-contraction: bf16 tensor_mul by tiled lin_w*sw1 + pairwise add tree
    20->10->5; the 10->5 level runs on the idle GPSIMD, the rest plus the
    EMA scans on DVE (2x bf16 perf mode), z-sigmoid on ACT.
  - No output copy: v EMA is stored unscaled, DMA'd straight out; the
    final *sw2 scale happens on host after the gather.
"""
import numpy as np
from contextlib import ExitStack
import sys

sys.path.insert(0, "/opt/trn_rl_repo")

import concourse.bass as bass
import concourse.bacc as bacc
import concourse.tile as tile
from concourse import mybir
from concourse.bass_utils import run_bass_kernel_spmd
import ml_dtypes

BF16 = ml_dtypes.bfloat16

B, F, T, H, K = 128, 34, 4000, 20, 5
NCORES = 8
S = 25           # outputs per block (500 cols = one PSUM bank)
JW = S + 4       # patch window (taps)
FA = F + 1       # augmented channels (x + ones)
ROWS = JW * FA   # 1015
NCHUNK = 10
CHROWS = 105     # 10*105 = 1050 (35 zero-pad rows)
NB = 21          # blocks per core
UGRP = 7         # blocks per h-contraction group
NG = NB // UGRP  # 3 groups
TL = NB * S      # 525
WARM = 25
TO = T // NCORES     # 500
PADL = 48
XT_W = 530
BCOLS = S * H        # 500
GT_T = UGRP * S      # 175 t per group
BN_EPS = 1e-5

_DT = mybir.dt

# (chunk, (col0, col1)) — per-block matmul list. start=True clears
# has_written for the WHOLE psum bank, so chunk 0 is the single start:
# full width, with every column's BN-shift bias on its ones row; chunks
# 1..9 accumulate their x-band column ranges.
MMLIST = [
    (0, (0, 500)),
    (1, (0, 120)),
    (2, (40, 180)),
    (3, (100, 240)),
    (4, (160, 300)),
    (5, (220, 360)),
    (6, (280, 420)),
    (7, (340, 480)),
    (8, (400, 500)),
    (9, (460, 500)),
]


def _sigmoid(v):
    return 1.0 / (1.0 + np.exp(-v))


def build_nc(sw1, sw2, reps=1):
    nc = bacc.Bacc()
    xt = nc.declare_dram_parameter("xt", [XT_W, FA, B], _DT.float8e3,
                                   isOutput=False)
    wcp = nc.declare_dram_parameter("wc", [CHROWS, NCHUNK, BCOLS], _DT.bfloat16,
                                    isOutput=False)
    wrp = nc.declare_dram_parameter("wrep", [B, UGRP * BCOLS], _DT.bfloat16,
                                    isOutput=False)
    cst = nc.declare_dram_parameter("consts", [B, 4], _DT.float32, isOutput=False)
    d0ap = nc.declare_dram_parameter("d0a", [B, TL], _DT.float32, isOutput=False)
    d0bp = nc.declare_dram_parameter("d0b", [B, TL], _DT.float32, isOutput=False)
    outp = nc.declare_dram_parameter("out", [B, TO], _DT.float32, isOutput=True)

    st_t = FA * B  # xt t-stride in elements

    with ExitStack() as ctx:
        tc = ctx.enter_context(tile.TileContext(nc))
        singles = ctx.enter_context(tc.tile_pool(name="singles", bufs=1))
        xp = ctx.enter_context(tc.tile_pool(name="xp", bufs=3))
        pp = ctx.enter_context(tc.tile_pool(name="pp", bufs=8, space="PSUM"))
        up = ctx.enter_context(tc.tile_pool(name="up", bufs=3))
        ump = ctx.enter_context(tc.tile_pool(name="ump", bufs=2))
        um2p = ctx.enter_context(tc.tile_pool(name="um2p", bufs=2))
        um3p = ctx.enter_context(tc.tile_pool(name="um3p", bufs=2))

        wc_sb = singles.tile([CHROWS, NCHUNK, BCOLS], _DT.bfloat16)
        nc.sync.dma_start(out=wc_sb, in_=wcp[:, :, :])
        wrep_sb = singles.tile([B, UGRP * BCOLS], _DT.bfloat16)
        nc.sync.dma_start(out=wrep_sb, in_=wrp[:, :])
        cst_sb = singles.tile([B, 4], _DT.float32)
        nc.sync.dma_start(out=cst_sb, in_=cst[:, :])
        d0a_sb = singles.tile([B, TL], _DT.float32)
        nc.sync.dma_start(out=d0a_sb, in_=d0ap[:, :])
        d0b_sb = singles.tile([B, TL], _DT.float32)
        nc.sync.dma_start(out=d0b_sb, in_=d0bp[:, :])

        pqzv = ctx.enter_context(tc.tile_pool(name="pqzv", bufs=2))

        xt_ap = xt[:, :, :]

        for _rep in range(reps):
            p_sb = pqzv.tile([B, TL], _DT.float32)
            q_sb = pqzv.tile([B, TL], _DT.float32)
            z_sb = pqzv.tile([B, TL], _DT.float32)
            v_sb = pqzv.tile([B, TL], _DT.float32)
            # one windowed DMA per 105-row chunk loads the whole rep's
            # lhsT patches: [105 rows, 21 blocks, 128 b] (~320 GB/s shape).
            xb = xp.tile([CHROWS, NCHUNK, NB, B], _DT.float8e3)
            for c in range(NCHUNK):
                src = bass.AP(
                    tensor=xt_ap.tensor,
                    offset=c * CHROWS * B,
                    ap=[[B, CHROWS], [S * st_t, NB], [1, B]],
                )
                eng = nc.sync if (c % 3 != 2) else nc.scalar
                eng.dma_start(out=xb[:, c, :, :], in_=src)

            for g in range(NG):
                u_g = up.tile([B, UGRP * BCOLS], _DT.bfloat16)
                for e in range(UGRP):
                    ibl = UGRP * g + e
                    psb = pp.tile([B, 512], _DT.float32)
                    for mi, (c, (a0, a1)) in enumerate(MMLIST):
                        nc.tensor.matmul(
                            psb[:, a0:a1], xb[:, c, ibl, :],
                            wc_sb[:, c, a0:a1],
                            start=(mi == 0), stop=(mi == len(MMLIST) - 1),
                            skip_group_check=True,
                        )
                    nc.scalar.activation(
                        out=u_g[:, BCOLS * e:BCOLS * (e + 1)],
                        in_=psb[:, 0:BCOLS],
                        func=mybir.ActivationFunctionType.Sigmoid,
                    )

                # h-contraction: p[:, t] = sum_h u*wrep; mul + 20->10 add
                # on DVE (2x bf16), 10->5 on Pool, 5->1 reduce on DVE.
                um = ump.tile([B, UGRP * BCOLS], _DT.bfloat16)
                nc.vector.tensor_mul(um[:, :], u_g[:, :], wrep_sb[:, :])
                umv = um.rearrange("p (t h) -> p t h", h=H)
                um2 = um2p.tile([B, GT_T * 10], _DT.bfloat16)
                um2v = um2.rearrange("p (t h) -> p t h", h=10)
                nc.vector.tensor_add(um2v[:, :, :], umv[:, :, 0:10],
                                     umv[:, :, 10:20])
                um3 = um3p.tile([B, GT_T * 5], _DT.bfloat16)
                um3v = um3.rearrange("p (t h) -> p t h", h=5)
                nc.gpsimd.tensor_add(um3v[:, :, :], um2v[:, :, 0:5],
                                     um2v[:, :, 5:10])
                nc.vector.tensor_reduce(
                    out=p_sb[:, GT_T * g:GT_T * (g + 1)],
                    in_=um3v[:, :, :],
                    axis=mybir.AxisListType.X, op=mybir.AluOpType.add,
                )

                # q-scan right after the reduce (its deps are fresh); the
                # z-sigmoid and v-scan are DEFERRED by 1 / 2 groups so they
                # never head-of-line-block the in-order ACT / DVE queues
                # while waiting on the cross-engine contraction chain.
                s0, s1 = GT_T * g, GT_T * (g + 1)
                nc.vector.tensor_tensor_scan(
                    out=q_sb[:, s0:s1], data0=d0a_sb[:, s0:s1],
                    data1=p_sb[:, s0:s1],
                    initial=(0.0 if g == 0 else q_sb[:, s0 - 1:s0]),
                    op0=mybir.AluOpType.mult, op1=mybir.AluOpType.add,
                )
                if g >= 1:
                    _emit_z(nc, g - 1, q_sb, z_sb, cst_sb)
                if g >= 2:
                    _emit_v_out(nc, g - 2, z_sb, v_sb, d0b_sb, outp)
            _emit_z(nc, NG - 1, q_sb, z_sb, cst_sb)
            _emit_v_out(nc, NG - 2, z_sb, v_sb, d0b_sb, outp)
            _emit_v_out(nc, NG - 1, z_sb, v_sb, d0b_sb, outp)
    nc.compile()
    return nc


def _emit_z(nc, g, q_sb, z_sb, cst_sb):
    s0, s1 = GT_T * g, GT_T * (g + 1)
    nc.scalar.activation(
        out=z_sb[:, s0:s1], in_=q_sb[:, s0:s1],
        func=mybir.ActivationFunctionType.Sigmoid,
        bias=cst_sb[:, 2:3],
    )


def _emit_v_out(nc, g, z_sb, v_sb, d0b_sb, outp):
    s0, s1 = GT_T * g, GT_T * (g + 1)
    nc.vector.tensor_tensor_scan(
        out=v_sb[:, s0:s1], data0=d0b_sb[:, s0:s1],
        data1=z_sb[:, s0:s1],
        initial=(0.0 if g == 0 else v_sb[:, s0 - 1:s0]),
        op0=mybir.AluOpType.mult, op1=mybir.AluOpType.add,
    )
    c0 = max(0, s0 - WARM)
    c1 = min(TO, s1 - WARM)
    nc.sync.dma_start(out=outp[:, c0:c1],
                      in_=v_sb[:, WARM + c0:WARM + c1])


def prep(x, conv_w, conv_b, bn_gamma, bn_beta, bn_mean, bn_var,
         lin_w, lin_b, w1, w2):
    x = np.asarray(x, np.float32)
    inv = (np.asarray(bn_gamma, np.float32)
           / np.sqrt(np.asarray(bn_var, np.float32) + BN_EPS))
    shift = (np.asarray(conv_b, np.float32)
             - np.asarray(bn_mean, np.float32)) * inv \
        + np.asarray(bn_beta, np.float32)
    sw1 = float(_sigmoid(np.float32(np.asarray(w1))))
    sw2 = float(_sigmoid(np.float32(np.asarray(w2))))
    linb = float(np.asarray(lin_b, np.float32).reshape(-1)[0])
    lw = np.asarray(lin_w, np.float32).reshape(-1)

    GT = PADL + T + 40
    x_aug = np.zeros((GT, FA, B), np.float32)
    x_aug[PADL:PADL + T, :F, :] = x[:, 0].transpose(2, 1, 0)
    x_aug[PADL:PADL + T, F, :] = 1.0
    x_aug_f8 = x_aug.astype(ml_dtypes.float8_e3m4)

    cw = np.asarray(conv_w, np.float32)[:, 0]  # [H,F,K]
    Wf = np.zeros((NCHUNK * CHROWS, BCOLS), np.float32)
    for i in range(S):
        for k in range(K):
            j = i + k
            Wf[j * FA:j * FA + F, i * H:(i + 1) * H] = \
                (cw[:, :, k] * inv[:, None]).T
        # all BN-shift biases on chunk-0's j=2 ones row (row 104): keeps the
        # single full-width start=True on chunk 0, and t=g0+25*ibl+2 stays
        # inside the real ones region for every core/block that matters.
        Wf[2 * FA + F, i * H:(i + 1) * H] = shift
    wc = np.ascontiguousarray(
        Wf.reshape(NCHUNK, CHROWS, BCOLS).transpose(1, 0, 2)).astype(BF16)

    wr = np.tile(lw * sw1, UGRP * S).astype(BF16)
    wrep = np.ascontiguousarray(np.broadcast_to(wr, (B, UGRP * BCOLS)))

    consts = np.zeros((B, 4), np.float32)
    consts[:, 0] = 1.0 - sw1
    consts[:, 1] = 1.0 - sw2
    consts[:, 2] = linb

    d0a = np.full((B, TL), 1.0 - sw1, np.float32)
    d0b = np.full((B, TL), 1.0 - sw2, np.float32)
    d0a0 = d0a.copy(); d0a0[:, WARM] = 0.0
    d0b0 = d0b.copy(); d0b0[:, WARM] = 0.0

    in_maps = []
    for c in range(NCORES):
        g0 = 500 * c + PADL - WARM - 2
        xtc = np.ascontiguousarray(x_aug_f8[g0:g0 + XT_W, :, :])
        in_maps.append({"xt": xtc, "wc": wc, "wrep": wrep, "consts": consts,
                        "d0a": d0a0 if c == 0 else d0a,
                        "d0b": d0b0 if c == 0 else d0b})
    return in_maps, sw1, sw2


_NC_CACHE = {}


def kernel(**inputs):
    in_maps, sw1, sw2 = prep(**inputs)
    key = (round(sw1, 9), round(sw2, 9))
    if key not in _NC_CACHE:
        _NC_CACHE[key] = build_nc(sw1, sw2)
    nc = _NC_CACHE[key]
    res = run_bass_kernel_spmd(nc, in_maps, list(range(NCORES)))
    outs = [np.asarray(res.results[c]["out"], np.float32)
            for c in range(NCORES)]
    return np.float32(sw2) * np.concatenate(outs, axis=1)
